# revision 2
# baseline (speedup 1.0000x reference)
"""Trainium2 Bass kernel for nn_EncoderText (4-layer SRU text encoder).

Reference computation:
  e = embed[x]                       # [B, T, K]
  4x SRU layers over time (layer0: k=4 projections incl highway; 1-3: k=3)
  gather last valid timestep per sequence, L2-normalize over features.

Strategy:
- Data-parallel over sequences across 8 NeuronCores, no collectives.
- Only VALID tokens are computed: sequences are LPT-bin-packed into
  NCH chunks of CCH columns per core (all cores share one program
  shape; the assignment of sequences to cores/chunks is host-side).
- Activations are channel-major [D, tokens]; the SRU recurrence
  c_t = f_t*c_{t-1} + (1-f_t)*cand_t maps onto the DVE
  tensor_tensor_scan along the free axis; a boundary mask (runtime
  input) zeroes f at each sequence start so sequences chain safely
  through one scan per column-group.
- Weights/activations/embeddings in bf16 (halves HBM traffic, full PE
  rate); recurrence and gates stay fp32.
- Chunks are fused into column groups of <=512 so each matmul streams
  the whole group (fewer, longer matmuls; one PSUM bank per gate).
- Last-timestep selection via PE: transpose the final h tiles and
  multiply with a host-built one-hot matrix [tokens, slot] -> the
  output lands directly as [slot, channels]; slot->sequence mapping is
  undone on the host.
"""

from contextlib import ExitStack

import numpy as np

import concourse.bass as bass
import concourse.mybir as mybir
import concourse.tile as tile
from concourse import bacc
from concourse.bass_utils import run_bass_kernel_spmd
from concourse.masks import make_identity

FP32 = mybir.dt.float32
BF16 = mybir.dt.bfloat16
I32 = mybir.dt.int32
AF = mybir.ActivationFunctionType
OP = mybir.AluOpType

BF16_NP = mybir.dt.np(BF16)

P = 128
N_CORES = 8
GW_MAX = 512  # max columns per matmul / PSUM bank (fp32)


def _ptiles(n):
    """[(start, size)] partition tiles of <=128 covering n."""
    out = []
    s = 0
    while s < n:
        out.append((s, min(P, n - s)))
        s += P
    return out


def _groups(NCH, CCH):
    """Fuse chunks into column groups of width <= GW_MAX."""
    cpg = max(1, GW_MAX // CCH)
    out = []
    c = 0
    while c < NCH:
        n = min(cpg, NCH - c)
        out.append((c * CCH, n * CCH))
        c += n
    return out


# ---------------------------------------------------------------------------
# Host-side packing plan
# ---------------------------------------------------------------------------


class Plan:
    def __init__(self, NCH, CCH, SMAX, bins):
        self.NCH = NCH  # chunks per core
        self.CCH = CCH  # columns per chunk
        self.SMAX = SMAX  # max sequences per chunk
        self.bins = bins  # [N_CORES][NCH] -> list of global seq ids
        self.NTOT = NCH * CCH
        self.NSLOT = NCH * SMAX


def make_plan(lengths):
    """LPT bin-pack sequences into N_CORES*NCH bins of capacity CCH."""
    lengths = np.asarray(lengths, np.int64)
    order = np.argsort(-lengths, kind="stable")
    maxlen = int(lengths.max())

    best = None
    for NCH in range(1, 17):
        nbins = N_CORES * NCH
        loads = np.zeros(nbins, np.int64)
        bins = [[] for _ in range(nbins)]
        for i in order:
            j = int(np.argmin(loads))
            loads[j] += int(lengths[i])
            bins[j].append(int(i))
        C = int(loads.max())
        CCH = max(maxlen, -(-C // 8) * 8)
        if CCH > GW_MAX:
            continue
        cost = NCH * CCH
        if best is None or cost < best[0]:
            SMAX = max(len(b) for b in bins)
            best = (cost, NCH, CCH, SMAX, bins)
        if best is not None and NCH * maxlen >= best[0]:
            break
    assert best is not None, "no feasible packing"
    _, NCH, CCH, SMAX, bins = best
    core_bins = [bins[c * NCH : (c + 1) * NCH] for c in range(N_CORES)]
    plan = Plan(NCH, CCH, SMAX, core_bins)
    assert plan.NSLOT <= P, f"too many slots {plan.NSLOT}"
    return plan


# ---------------------------------------------------------------------------
# Device program
# ---------------------------------------------------------------------------


def build_program(K, D, V, NCH, CCH, SMAX):
    """Emit the per-core program.

    DRAM parameters (per core):
      xidx   [NTOKP]      int32  packed token row indices (pad -> 0)
      embed  [V, K]       bf16   full embedding table (replicated)
      W{l}t  [NKD*gates, P, nki*P] bf16  host-pre-tiled weights
      bf{l}, br{l} [128, NKD] f32  per-channel-tile bias columns
      bmask  [128, NTOT]  f32    0 at each sequence start (and padding), else 1
      onehot [128, NJ*NSLOT] bf16  one-hot last-token selector per slot
      out    [NSLOT, D]   f32    normalized last-step hidden per slot
    """
    NTOT = NCH * CCH
    NSLOT = NCH * SMAX
    NTOKP = ((NTOT + P - 1) // P) * P
    NJ = NTOKP // P
    KT_IN = _ptiles(K)
    KT_D = _ptiles(D)
    NKI, NKD = len(KT_IN), len(KT_D)
    TOK_T = _ptiles(NTOT)
    GRP = _groups(NCH, CCH)
    assert max(gw for _, gw in GRP) <= GW_MAX

    nc = bacc.Bacc("TRN2", target_bir_lowering=False, debug=False)

    xidx = nc.declare_dram_parameter("xidx", [NTOKP], I32, isOutput=False)
    emb = nc.declare_dram_parameter("embed", [V, K], BF16, isOutput=False)
    Wd = []
    for l in range(4):
        gates_l = 4 if l == 0 else 3
        nki_l = NKI if l == 0 else NKD
        Wd.append(
            nc.declare_dram_parameter(
                f"W{l}t", [NKD * gates_l, P, nki_l * P], BF16, isOutput=False
            )
        )
    bfd = [
        nc.declare_dram_parameter(f"bf{l}", [P, NKD], FP32, isOutput=False)
        for l in range(4)
    ]
    brd = [
        nc.declare_dram_parameter(f"br{l}", [P, NKD], FP32, isOutput=False)
        for l in range(4)
    ]
    bmask_d = nc.declare_dram_parameter("bmask", [P, NTOT], FP32, isOutput=False)
    onehot_d = nc.declare_dram_parameter(
        "onehot", [P, NJ * NSLOT], BF16, isOutput=False
    )
    out_d = nc.declare_dram_parameter("out", [NSLOT, D], FP32, isOutput=True)

    with tile.TileContext(nc) as tc, ExitStack() as ctx:
        sb = ctx.enter_context(tc.tile_pool(name="sb", bufs=1))
        big = ctx.enter_context(tc.tile_pool(name="big", bufs=NKI + 2 * NKD))
        wp = ctx.enter_context(tc.tile_pool(name="wp", bufs=1))
        tp = ctx.enter_context(tc.tile_pool(name="tp", bufs=2))
        pp = ctx.enter_context(tc.tile_pool(name="pp", bufs=1, space="PSUM"))

        # ---- constants ----
        identity = sb.tile([P, P], BF16, tag="identity")
        make_identity(nc, identity[:])
        bmask = sb.tile([P, NTOT], FP32, tag="bmask")
        nc.sync.dma_start(out=bmask[:], in_=bmask_d[:, :])
        onehot = sb.tile([P, NJ * NSLOT], BF16, tag="onehot")
        nc.sync.dma_start(out=onehot[:], in_=onehot_d[:, :])
        bfs, brs = [], []
        for l in range(4):
            bft = sb.tile([P, NKD], FP32, tag=f"bf{l}")
            nc.sync.dma_start(out=bft[:], in_=bfd[l][:, :])
            bfs.append(bft)
            brt = sb.tile([P, NKD], FP32, tag=f"br{l}")
            nc.sync.dma_start(out=brt[:], in_=brd[l][:, :])
            brs.append(brt)
        idx_sb = sb.tile([P, NJ], I32, tag="idx")
        nc.sync.dma_start(
            out=idx_sb[:], in_=xidx[:].rearrange("(j p) -> p j", p=P)
        )

        # ---- embedding gather + transpose to channel-major e_T (bf16) ----
        eT = []
        for k in range(NKI):
            et = big.tile([P, NTOT], BF16, tag="hb", name=f"eT{k}")
            eT.append(et)
        for j, (ts_, tj) in enumerate(TOK_T):
            eg = sb.tile([P, K], BF16, tag="eg", bufs=2, name=f"eg{j}")
            nc.gpsimd.indirect_dma_start(
                out=eg[:tj, :],
                out_offset=None,
                in_=emb[:, :],
                in_offset=bass.IndirectOffsetOnAxis(
                    ap=idx_sb[:tj, j : j + 1], axis=0
                ),
            )
            for k, (ks, kk) in enumerate(KT_IN):
                pt = pp.tile([P, P], BF16, tag="ptT", bufs=2, name=f"ptr{j}_{k}")
                nc.tensor.transpose(
                    out=pt[:kk, :tj],
                    in_=eg[:tj, ks : ks + kk],
                    identity=identity[:tj, :tj],
                )
                nc.vector.tensor_copy(
                    out=eT[k][:kk, ts_ : ts_ + tj], in_=pt[:kk, :tj]
                )

        # ---- SRU layers ----
        out_sb = sb.tile([NSLOT, D], FP32, tag="out_sb")

        in_tiles = eT
        ktin = KT_IN
        for l in range(4):
            gates = 4 if l == 0 else 3
            nki = len(ktin)

            new_h = None
            if l < 3:
                new_h = [
                    big.tile([P, NTOT], BF16, tag="hb", name=f"h{l}_{ci}")
                    for ci in range(NKD)
                ]

            for ci, (cs, mc) in enumerate(KT_D):
                wts = []
                for g in range(gates):
                    wt = wp.tile(
                        [P, nki, P],
                        BF16,
                        tag=f"w{g}",
                        bufs=3,
                        name=f"w{l}_{ci}_{g}",
                    )
                    nc.sync.dma_start(
                        out=wt[:, :, :],
                        in_=Wd[l][ci * gates + g, :, :].rearrange(
                            "p (kt m) -> p kt m", m=P
                        ),
                    )
                    wts.append(wt)

                hh = None
                if l == 3:
                    hh = tp.tile([P, NTOT], BF16, tag="hh", name=f"hh{ci}")

                for gs, gw in GRP:
                    gsl = slice(gs, gs + gw)
                    ps = []
                    for g in range(gates):
                        pt = pp.tile(
                            [P, GW_MAX], FP32, tag="pt", bufs=5,
                            name=f"pm{l}_{ci}_{g}",
                        )
                        for k2, (ks2, kk2) in enumerate(ktin):
                            nc.tensor.matmul(
                                out=pt[:mc, :gw],
                                lhsT=wts[g][:kk2, k2, :mc],
                                rhs=in_tiles[k2][:kk2, gsl],
                                start=(k2 == 0),
                                stop=(k2 == nki - 1),
                            )
                        ps.append(pt)
                    cand = ps[0][:mc, :gw]
                    fpre = ps[1][:mc, :gw]
                    rpre = ps[2][:mc, :gw]

                    fsb = tp.tile([P, GW_MAX], FP32, tag="fsb", name=f"f{l}_{ci}")
                    nc.scalar.activation(
                        out=fsb[:mc, :gw],
                        in_=fpre,
                        func=AF.Sigmoid,
                        bias=bfs[l][:mc, ci : ci + 1],
                    )
                    rsb = tp.tile([P, GW_MAX], FP32, tag="rsb", name=f"r{l}_{ci}")
                    nc.scalar.activation(
                        out=rsb[:mc, :gw],
                        in_=rpre,
                        func=AF.Sigmoid,
                        bias=brs[l][:mc, ci : ci + 1],
                    )
                    # z' = (f - 1) * cand  == -(1-f)*cand
                    zb = tp.tile([P, GW_MAX], FP32, tag="zb", name=f"z{l}_{ci}")
                    nc.vector.scalar_tensor_tensor(
                        out=zb[:mc, :gw],
                        in0=fsb[:mc, :gw],
                        scalar=1.0,
                        in1=cand,
                        op0=OP.subtract,
                        op1=OP.mult,
                    )
                    # f masked at sequence starts (in place)
                    nc.vector.tensor_mul(
                        out=fsb[:mc, :gw], in0=fsb[:mc, :gw], in1=bmask[:mc, gsl]
                    )
                    # c_t = fm*c_{t-1} - z'
                    cst = tp.tile([P, GW_MAX], FP32, tag="cst", name=f"c{l}_{ci}")
                    nc.vector.tensor_tensor_scan(
                        out=cst[:mc, :gw],
                        data0=fsb[:mc, :gw],
                        data1=zb[:mc, :gw],
                        initial=0.0,
                        op0=OP.mult,
                        op1=OP.subtract,
                    )
                    # h = r*tanh(c) + (1-r)*xres, built in place over cst
                    nc.scalar.activation(
                        out=cst[:mc, :gw], in_=cst[:mc, :gw], func=AF.Tanh
                    )
                    if l == 0:
                        xres = ps[3][:mc, :gw]
                    else:
                        xres = in_tiles[ci][:mc, gsl]
                    nc.vector.tensor_sub(
                        out=cst[:mc, :gw], in0=cst[:mc, :gw], in1=xres
                    )
                    nc.vector.tensor_mul(
                        out=cst[:mc, :gw], in0=cst[:mc, :gw], in1=rsb[:mc, :gw]
                    )
                    dest = new_h[ci] if l < 3 else hh
                    nc.vector.tensor_add(
                        out=dest[:mc, gsl], in0=cst[:mc, :gw], in1=xres
                    )

                if l == 3:
                    # last-token selection: sel[slot, ch] = onehot.T @ hh.T
                    hTs = []
                    for j, (ts_, tj) in enumerate(TOK_T):
                        ptj = pp.tile(
                            [P, P], BF16, tag="ptT", bufs=2, name=f"ptT{ci}_{j}"
                        )
                        nc.tensor.transpose(
                            out=ptj[:tj, :mc],
                            in_=hh[:mc, ts_ : ts_ + tj],
                            identity=identity[:mc, :mc],
                        )
                        hT = tp.tile(
                            [P, P], BF16, tag="hT", bufs=NJ, name=f"hT{ci}_{j}"
                        )
                        nc.vector.tensor_copy(
                            out=hT[:tj, :mc], in_=ptj[:tj, :mc]
                        )
                        hTs.append(hT)
                    sel = pp.tile(
                        [P, P], FP32, tag="sel", bufs=1, name=f"sel{ci}"
                    )
                    for j, (ts_, tj) in enumerate(TOK_T):
                        nc.tensor.matmul(
                            out=sel[:NSLOT, :mc],
                            lhsT=onehot[:tj, j * NSLOT : (j + 1) * NSLOT],
                            rhs=hTs[j][:tj, :mc],
                            start=(j == 0),
                            stop=(j == NJ - 1),
                        )
                    nc.vector.tensor_copy(
                        out=out_sb[:, cs : cs + mc], in_=sel[:NSLOT, :mc]
                    )

            if l < 3:
                in_tiles = new_h
                ktin = KT_D

        # ---- epilogue: L2-normalize selected hidden, write out ----
        nq = (D + GW_MAX - 1) // GW_MAX
        ssp = sb.tile([NSLOT, nq], FP32, tag="ssp")
        for q in range(nq):
            q0 = q * GW_MAX
            qw = min(GW_MAX, D - q0)
            pt = pp.tile([P, GW_MAX], FP32, tag="pt", bufs=5, name=f"ptq{q}")
            nc.scalar.activation(
                out=pt[:NSLOT, :qw],
                in_=out_sb[:, q0 : q0 + qw],
                func=AF.Square,
                accum_out=ssp[:, q : q + 1],
            )
        ss = sb.tile([NSLOT, 1], FP32, tag="ss")
        nc.vector.tensor_reduce(
            out=ss[:], in_=ssp[:], axis=mybir.AxisListType.X, op=OP.add
        )
        # 1/sqrt(ss + eps); eps guards empty slots (zero rows)
        eps = sb.tile([NSLOT, 1], FP32, tag="eps")
        nc.gpsimd.memset(eps[:], 1e-20)
        sq = sb.tile([NSLOT, 1], FP32, tag="sq")
        nc.scalar.activation(out=sq[:], in_=ss[:], func=AF.Sqrt, bias=eps[:, 0:1])
        inv = sb.tile([NSLOT, 1], FP32, tag="inv")
        nc.vector.reciprocal(out=inv[:], in_=sq[:])
        nc.vector.tensor_scalar(
            out=out_sb[:], in0=out_sb[:], scalar1=inv[:, 0:1], scalar2=None,
            op0=OP.mult,
        )
        nc.sync.dma_start(out=out_d[:, :], in_=out_sb[:])

    nc.compile()
    return nc


# ---------------------------------------------------------------------------
# Host-side input prep
# ---------------------------------------------------------------------------


def _retile_W(W, Kin, D, gates, NKD, NKI_l):
    """[Kin, gates*D] -> [NKD*gates, 128, nki*128] per-partition-contiguous."""
    nki = NKI_l
    Wp = np.zeros((nki * P, gates * NKD * P), np.float32)
    Dp = NKD * P
    src = np.asarray(W, np.float32)
    for g in range(gates):
        Wp[:Kin, g * Dp : g * Dp + D] = src[:, g * D : (g + 1) * D]
    # [kt*P+p, g*Dp + ci*P + m] -> [ci*gates+g, p, kt*P+m]
    Wp = Wp.reshape(nki, P, gates, NKD, P)
    Wt = np.ascontiguousarray(np.transpose(Wp, (3, 2, 1, 0, 4)))
    return Wt.reshape(NKD * gates, P, nki * P).astype(BF16_NP)


def _pack_bias(b_half, D, NKD):
    """[D] -> [128, NKD]: column ci holds channels ci*128..ci*128+127."""
    pad = NKD * P - D
    bp = np.pad(np.asarray(b_half, np.float32), (0, pad))
    return np.ascontiguousarray(bp.reshape(NKD, P).T)


def make_core_inputs(core, plan, x, lengths, embed, Ws, bs, K, D, V):
    NCH, CCH, SMAX = plan.NCH, plan.CCH, plan.SMAX
    NTOT, NSLOT = plan.NTOT, plan.NSLOT
    NTOKP = ((NTOT + P - 1) // P) * P
    NJ = NTOKP // P
    NKD = len(_ptiles(D))

    xl = np.zeros(NTOKP, np.int32)
    bmask = np.zeros((1, NTOT), np.float32)
    onehot = np.zeros((NTOKP, NSLOT), np.float32)
    for ch, bin_seqs in enumerate(plan.bins[core]):
        pos = 0
        for k, b in enumerate(bin_seqs):
            ln = int(lengths[b])
            if ln <= 0:
                continue
            col0 = ch * CCH + pos
            xl[col0 : col0 + ln] = x[b, :ln]
            bmask[0, col0 + 1 : col0 + ln] = 1.0
            onehot[col0 + ln - 1, ch * SMAX + k] = 1.0
            pos += ln

    # [NTOKP, NSLOT] -> [P, NJ*NSLOT]: chunk j partition p = token j*128+p
    oh = np.ascontiguousarray(
        onehot.reshape(NJ, P, NSLOT).transpose(1, 0, 2).reshape(P, NJ * NSLOT)
    ).astype(BF16_NP)

    emb_bf = embed if embed.dtype == BF16_NP else np.asarray(embed).astype(BF16_NP)

    im = {
        "xidx": xl,
        "embed": emb_bf,
        "bmask": np.broadcast_to(bmask, (P, NTOT)).copy(),
        "onehot": oh,
    }
    for l in range(4):
        im[f"W{l}t"] = Ws[l]
        im[f"bf{l}"] = _pack_bias(bs[l][:D], D, NKD)
        im[f"br{l}"] = _pack_bias(bs[l][D:], D, NKD)
    return im


_NC_CACHE = {}


def kernel(x, lengths, embed, W0, b0, W1, b1, W2, b2, W3, b3):
    x = np.asarray(x)
    lengths = np.asarray(lengths)
    embed = np.asarray(embed, np.float32).astype(BF16_NP)
    Ws = [np.asarray(w, np.float32) for w in (W0, W1, W2, W3)]
    bs = [np.asarray(b, np.float32) for b in (b0, b1, b2, b3)]

    Bb, T = x.shape
    V, K = embed.shape
    D = Ws[1].shape[0]

    plan = make_plan(lengths)
    key = (K, D, V, plan.NCH, plan.CCH, plan.SMAX)
    if key not in _NC_CACHE:
        _NC_CACHE[key] = build_program(*key)
    nc = _NC_CACHE[key]

    NKD = len(_ptiles(D))
    NKI = len(_ptiles(K))
    Wt = [
        _retile_W(Ws[0], K, D, 4, NKD, NKI),
        _retile_W(Ws[1], D, D, 3, NKD, NKD),
        _retile_W(Ws[2], D, D, 3, NKD, NKD),
        _retile_W(Ws[3], D, D, 3, NKD, NKD),
    ]
    in_maps = [
        make_core_inputs(c, plan, x, lengths, embed, Wt, bs, K, D, V)
        for c in range(N_CORES)
    ]
    res = run_bass_kernel_spmd(nc, in_maps, core_ids=list(range(N_CORES)))

    out = np.zeros((Bb, D), np.float32)
    for c in range(N_CORES):
        oc = res.results[c]["out"]
        for ch, bin_seqs in enumerate(plan.bins[c]):
            for k, b in enumerate(bin_seqs):
                out[b] = oc[ch * plan.SMAX + k]
    return out


# revision 12
# speedup vs baseline: 1.5318x; 1.5318x over previous
"""Trainium2 Bass kernel for nn_EncoderText (4-layer SRU text encoder).

Reference computation:
  e = embed[x]                       # [B, T, K]
  4x SRU layers over time (layer0: k=4 projections incl highway; 1-3: k=3)
  gather last valid timestep per sequence, L2-normalize over features.

Strategy:
- Data-parallel over sequences across 8 NeuronCores, no collectives.
- Only VALID tokens are computed: sequences are LPT-bin-packed into
  NCH chunks of CCH columns per core (all cores share one program
  shape; the assignment of sequences to cores/chunks is host-side).
- Activations are channel-major [D, tokens]; the SRU recurrence
  c_t = f_t*c_{t-1} + (1-f_t)*cand_t maps onto the DVE
  tensor_tensor_scan along the free axis; a boundary mask (runtime
  input) zeroes f at each sequence start so sequences chain safely
  through one scan per column-group.
- Candidate/highway weights+activations in bf16; f/r gate projections
  in fp8e4m3 (weights x64, activations x16, descaled inside the
  sigmoid) using DoubleRow matmuls that contract two k-tiles per
  instruction; recurrence and gates stay fp32.
- Chunks are fused into column groups of <=512 so each matmul streams
  the whole group (fewer, longer matmuls; one PSUM bank per gate).
- Last-timestep selection via PE: transpose the final h tiles and
  multiply with a host-built one-hot matrix [tokens, slot] -> the
  output lands directly as [slot, channels]; slot->sequence mapping is
  undone on the host.
"""

from contextlib import ExitStack

import numpy as np

import concourse.bass as bass
import concourse.mybir as mybir
import concourse.tile as tile
from concourse import bacc
from concourse.bass_utils import run_bass_kernel_spmd
from concourse.masks import make_identity

FP32 = mybir.dt.float32
BF16 = mybir.dt.bfloat16
FP8 = mybir.dt.float8e4
I32 = mybir.dt.int32
AF = mybir.ActivationFunctionType
OP = mybir.AluOpType
DR = mybir.MatmulPerfMode.DoubleRow

BF16_NP = mybir.dt.np(BF16)
FP8_NP = mybir.dt.np(FP8)

P = 128
N_CORES = 8
GW_MAX = 512  # max columns per matmul / PSUM bank (fp32)
W8S = 64.0  # fp8 gate-weight scale
H8S = 16.0  # fp8 gate-activation scale
SIG_SCALE = 1.0 / (W8S * H8S)


def _ptiles(n):
    """[(start, size)] partition tiles of <=128 covering n."""
    out = []
    s = 0
    while s < n:
        out.append((s, min(P, n - s)))
        s += P
    return out


def _groups(NCH, CCH):
    """Fuse chunks into column groups of width <= GW_MAX."""
    cpg = max(1, GW_MAX // CCH)
    out = []
    c = 0
    while c < NCH:
        n = min(cpg, NCH - c)
        out.append((c * CCH, n * CCH))
        c += n
    return out


# ---------------------------------------------------------------------------
# Host-side packing plan
# ---------------------------------------------------------------------------


class Plan:
    def __init__(self, NCH, CCH, SMAX, bins):
        self.NCH = NCH  # chunks per core
        self.CCH = CCH  # columns per chunk
        self.SMAX = SMAX  # max sequences per chunk
        self.bins = bins  # [N_CORES][NCH] -> list of global seq ids
        self.NTOT = NCH * CCH
        self.NSLOT = NCH * SMAX


def make_plan(lengths):
    """LPT bin-pack sequences into N_CORES*NCH bins of capacity CCH."""
    lengths = np.asarray(lengths, np.int64)
    order = np.argsort(-lengths, kind="stable")
    maxlen = int(lengths.max())

    best = None
    for NCH in range(1, 17):
        nbins = N_CORES * NCH
        loads = np.zeros(nbins, np.int64)
        bins = [[] for _ in range(nbins)]
        for i in order:
            j = int(np.argmin(loads))
            loads[j] += int(lengths[i])
            bins[j].append(int(i))
        C = int(loads.max())
        CCH = max(maxlen, -(-C // 8) * 8)
        if CCH > GW_MAX:
            continue
        cost = NCH * CCH
        if best is None or cost < best[0]:
            SMAX = max(len(b) for b in bins)
            best = (cost, NCH, CCH, SMAX, bins)
        if best is not None and NCH * maxlen >= best[0]:
            break
    assert best is not None, "no feasible packing"
    _, NCH, CCH, SMAX, bins = best
    core_bins = [bins[c * NCH : (c + 1) * NCH] for c in range(N_CORES)]
    plan = Plan(NCH, CCH, SMAX, core_bins)
    assert plan.NSLOT <= P, f"too many slots {plan.NSLOT}"
    return plan


# ---------------------------------------------------------------------------
# Device program
# ---------------------------------------------------------------------------


def build_program(K, D, V, NCH, CCH, SMAX):
    """Emit the per-core program.

    DRAM parameters (per core):
      xidx   [NTOKP]      int32  packed token row indices (pad -> 0)
      embed  [V, K]       bf16   full embedding table (replicated)
      W{l}t  [NKD*gates, P, nki*P] bf16  host-pre-tiled weights
      bf{l}, br{l} [128, NKD] f32  per-channel-tile bias columns
      bmask  [128, NTOT]  f32    0 at each sequence start (and padding), else 1
      onehot [128, NJ*NSLOT] bf16  one-hot last-token selector per slot
      out    [NSLOT, D]   f32    normalized last-step hidden per slot
    """
    NTOT = NCH * CCH
    NSLOT = NCH * SMAX
    NTOKP = ((NTOT + P - 1) // P) * P
    NJ = NTOKP // P
    KT_IN = _ptiles(K)
    KT_D = _ptiles(D)
    NKI, NKD = len(KT_IN), len(KT_D)
    TOK_T = _ptiles(NTOT)
    GRP = _groups(NCH, CCH)
    assert max(gw for _, gw in GRP) <= GW_MAX

    nc = bacc.Bacc("TRN2", target_bir_lowering=False, debug=False)

    xidx = nc.declare_dram_parameter("xidx", [NTOKP], I32, isOutput=False)
    emb = nc.declare_dram_parameter("embed", [V, K], BF16, isOutput=False)
    Wd, Wd8 = [], []
    for l in range(4):
        gb = 2 if l == 0 else 1  # bf16 gates: cand (+ highway on l0)
        nki_l = NKI if l == 0 else NKD
        Wd.append(
            nc.declare_dram_parameter(
                f"W{l}t", [NKD * gb, P, nki_l * P], BF16, isOutput=False
            )
        )
        Wd8.append(
            nc.declare_dram_parameter(
                f"W{l}g8", [NKD * 2, P, nki_l * P], FP8, isOutput=False
            )
        )
    bfd = [
        nc.declare_dram_parameter(f"bf{l}", [P, NKD], FP32, isOutput=False)
        for l in range(4)
    ]
    brd = [
        nc.declare_dram_parameter(f"br{l}", [P, NKD], FP32, isOutput=False)
        for l in range(4)
    ]
    bmask_d = nc.declare_dram_parameter("bmask", [P, NTOT], FP32, isOutput=False)
    onehot_d = nc.declare_dram_parameter(
        "onehot", [P, NJ * NSLOT], BF16, isOutput=False
    )
    out_d = nc.declare_dram_parameter("out", [NSLOT, D], FP32, isOutput=True)

    with tile.TileContext(nc) as tc, ExitStack() as ctx:
        sb = ctx.enter_context(tc.tile_pool(name="sb", bufs=1))
        big = ctx.enter_context(tc.tile_pool(name="big", bufs=NKI + 2 * NKD))
        wp = ctx.enter_context(tc.tile_pool(name="wp", bufs=1))
        tp = ctx.enter_context(tc.tile_pool(name="tp", bufs=2))
        pp = ctx.enter_context(tc.tile_pool(name="pp", bufs=1, space="PSUM"))

        # ---- constants ----
        identity = sb.tile([P, P], BF16, tag="identity")
        make_identity(nc, identity[:])
        bmask = sb.tile([P, NTOT], FP32, tag="bmask")
        nc.sync.dma_start(out=bmask[:], in_=bmask_d[:, :])
        onehot = sb.tile([P, NJ * NSLOT], BF16, tag="onehot")
        nc.sync.dma_start(out=onehot[:], in_=onehot_d[:, :])
        bfs, brs = [], []
        for l in range(4):
            bft = sb.tile([P, NKD], FP32, tag=f"bf{l}")
            nc.sync.dma_start(out=bft[:], in_=bfd[l][:, :])
            bfs.append(bft)
            brt = sb.tile([P, NKD], FP32, tag=f"br{l}")
            nc.sync.dma_start(out=brt[:], in_=brd[l][:, :])
            brs.append(brt)
        idx_sb = sb.tile([P, NJ], I32, tag="idx")
        nc.sync.dma_start(
            out=idx_sb[:], in_=xidx[:].rearrange("(j p) -> p j", p=P)
        )

        # ---- embedding gather + transpose to channel-major e_T (bf16+fp8) ----
        eT = []
        for k in range(NKI):
            et = big.tile([P, NTOT], BF16, tag="hb", name=f"eT{k}")
            eT.append(et)
        e8 = sb.tile([P, NKI, NTOT], FP8, tag="e8")
        for j, (ts_, tj) in enumerate(TOK_T):
            eg = sb.tile([P, K], BF16, tag="eg", bufs=2, name=f"eg{j}")
            nc.gpsimd.indirect_dma_start(
                out=eg[:tj, :],
                out_offset=None,
                in_=emb[:, :],
                in_offset=bass.IndirectOffsetOnAxis(
                    ap=idx_sb[:tj, j : j + 1], axis=0
                ),
            )
            for k, (ks, kk) in enumerate(KT_IN):
                pt = pp.tile([P, P], BF16, tag="ptT", bufs=2, name=f"ptr{j}_{k}")
                nc.tensor.transpose(
                    out=pt[:kk, :tj],
                    in_=eg[:tj, ks : ks + kk],
                    identity=identity[:tj, :tj],
                )
                nc.vector.tensor_copy(
                    out=eT[k][:kk, ts_ : ts_ + tj], in_=pt[:kk, :tj]
                )
                nc.vector.tensor_scalar(
                    out=e8[:kk, k, ts_ : ts_ + tj], in0=pt[:kk, :tj],
                    scalar1=H8S, scalar2=None, op0=OP.mult,
                )

        # ---- SRU layers ----
        out_sb = sb.tile([NSLOT, D], FP32, tag="out_sb")

        in_tiles = eT
        in8 = e8
        ktin = KT_IN
        for l in range(4):
            gb = 2 if l == 0 else 1
            nki = len(ktin)
            npair = nki // 2
            krem = ktin[-1][1] if nki % 2 else None

            new_h = None
            h8n = None
            if l < 3:
                new_h = [
                    big.tile([P, NTOT], BF16, tag="hb", name=f"h{l}_{ci}")
                    for ci in range(NKD)
                ]
                h8n = tp.tile([P, NKD, NTOT], FP8, tag="h8", bufs=2,
                              name=f"h8_{l}")

            for ci, (cs, mc) in enumerate(KT_D):
                wts = []
                for g in range(gb):
                    wt = wp.tile(
                        [P, nki, P], BF16, tag=f"wb{g}", bufs=3,
                        name=f"w{l}_{ci}_{g}",
                    )
                    nc.sync.dma_start(
                        out=wt[:, :, :],
                        in_=Wd[l][ci * gb + g, :, :].rearrange(
                            "p (kt m) -> p kt m", m=P
                        ),
                    )
                    wts.append(wt)
                w8s = []
                for g in range(2):
                    w8 = wp.tile(
                        [P, nki, P], FP8, tag=f"w8{g}", bufs=3,
                        name=f"w8_{l}_{ci}_{g}",
                    )
                    nc.sync.dma_start(
                        out=w8[:, :, :],
                        in_=Wd8[l][ci * 2 + g, :, :].rearrange(
                            "p (kt m) -> p kt m", m=P
                        ),
                    )
                    w8s.append(w8)

                hh = None
                if l == 3:
                    hh = tp.tile([P, NTOT], BF16, tag="hh", name=f"hh{ci}")

                for gs, gw in GRP:
                    gsl = slice(gs, gs + gw)
                    # bf16 gates: cand (+ highway)
                    ps = []
                    for g in range(gb):
                        pt = pp.tile(
                            [P, GW_MAX], FP32, tag="pt", bufs=5,
                            name=f"pm{l}_{ci}_{g}",
                        )
                        for k2, (ks2, kk2) in enumerate(ktin):
                            nc.tensor.matmul(
                                out=pt[:mc, :gw],
                                lhsT=wts[g][:kk2, k2, :mc],
                                rhs=in_tiles[k2][:kk2, gsl],
                                start=(k2 == 0),
                                stop=(k2 == nki - 1),
                            )
                        ps.append(pt)
                    # fp8 gates: f, r via DoubleRow k-tile pairs
                    ps8 = []
                    for g in range(2):
                        pt = pp.tile(
                            [P, GW_MAX], FP32, tag="pt", bufs=5,
                            name=f"pg{l}_{ci}_{g}",
                        )
                        for j in range(npair):
                            nc.tensor.matmul(
                                out=pt[:mc, :gw],
                                lhsT=w8s[g][:, 2 * j : 2 * j + 2, :mc],
                                rhs=in8[:, 2 * j : 2 * j + 2, gsl],
                                start=(j == 0),
                                stop=(j == npair - 1 and krem is None),
                                perf_mode=DR,
                            )
                        if krem is not None:
                            nc.tensor.matmul(
                                out=pt[:mc, :gw],
                                lhsT=w8s[g][:krem, nki - 1, :mc],
                                rhs=in8[:krem, nki - 1, gsl],
                                start=(npair == 0),
                                stop=True,
                            )
                        ps8.append(pt)
                    cand = ps[0][:mc, :gw]
                    fpre = ps8[0][:mc, :gw]
                    rpre = ps8[1][:mc, :gw]

                    fsb = tp.tile([P, GW_MAX], FP32, tag="fsb", name=f"f{l}_{ci}")
                    nc.scalar.activation(
                        out=fsb[:mc, :gw],
                        in_=fpre,
                        func=AF.Sigmoid,
                        bias=bfs[l][:mc, ci : ci + 1],
                        scale=SIG_SCALE,
                    )
                    rsb = tp.tile([P, GW_MAX], FP32, tag="rsb", name=f"r{l}_{ci}")
                    nc.scalar.activation(
                        out=rsb[:mc, :gw],
                        in_=rpre,
                        func=AF.Sigmoid,
                        bias=brs[l][:mc, ci : ci + 1],
                        scale=SIG_SCALE,
                    )
                    # z' = (f - 1) * cand  == -(1-f)*cand
                    zb = tp.tile([P, GW_MAX], FP32, tag="zb", name=f"z{l}_{ci}")
                    nc.vector.scalar_tensor_tensor(
                        out=zb[:mc, :gw],
                        in0=fsb[:mc, :gw],
                        scalar=1.0,
                        in1=cand,
                        op0=OP.subtract,
                        op1=OP.mult,
                    )
                    # f masked at sequence starts (in place)
                    nc.vector.tensor_mul(
                        out=fsb[:mc, :gw], in0=fsb[:mc, :gw], in1=bmask[:mc, gsl]
                    )
                    # c_t = fm*c_{t-1} - z'
                    cst = tp.tile([P, GW_MAX], FP32, tag="cst", name=f"c{l}_{ci}")
                    nc.vector.tensor_tensor_scan(
                        out=cst[:mc, :gw],
                        data0=fsb[:mc, :gw],
                        data1=zb[:mc, :gw],
                        initial=0.0,
                        op0=OP.mult,
                        op1=OP.subtract,
                    )
                    # h = r*tanh(c) + (1-r)*xres, built in place over cst
                    nc.scalar.activation(
                        out=cst[:mc, :gw], in_=cst[:mc, :gw], func=AF.Tanh
                    )
                    if l == 0:
                        xres = ps[1][:mc, :gw]
                    else:
                        xres = in_tiles[ci][:mc, gsl]
                    nc.vector.tensor_sub(
                        out=cst[:mc, :gw], in0=cst[:mc, :gw], in1=xres
                    )
                    nc.vector.tensor_mul(
                        out=cst[:mc, :gw], in0=cst[:mc, :gw], in1=rsb[:mc, :gw]
                    )
                    dest = new_h[ci] if l < 3 else hh
                    nc.vector.tensor_add(
                        out=dest[:mc, gsl], in0=cst[:mc, :gw], in1=xres
                    )
                    if l < 3:
                        nc.vector.tensor_scalar(
                            out=h8n[:mc, ci, gsl], in0=dest[:mc, gsl],
                            scalar1=H8S, scalar2=None, op0=OP.mult,
                        )

                if l == 3:
                    # last-token selection: sel[slot, ch] = onehot.T @ hh.T
                    hTs = []
                    for j, (ts_, tj) in enumerate(TOK_T):
                        ptj = pp.tile(
                            [P, P], BF16, tag="ptT", bufs=2, name=f"ptT{ci}_{j}"
                        )
                        nc.tensor.transpose(
                            out=ptj[:tj, :mc],
                            in_=hh[:mc, ts_ : ts_ + tj],
                            identity=identity[:mc, :mc],
                        )
                        hT = tp.tile(
                            [P, P], BF16, tag="hT", bufs=NJ, name=f"hT{ci}_{j}"
                        )
                        nc.vector.tensor_copy(
                            out=hT[:tj, :mc], in_=ptj[:tj, :mc]
                        )
                        hTs.append(hT)
                    sel = pp.tile(
                        [P, P], FP32, tag="sel", bufs=1, name=f"sel{ci}"
                    )
                    for j, (ts_, tj) in enumerate(TOK_T):
                        nc.tensor.matmul(
                            out=sel[:NSLOT, :mc],
                            lhsT=onehot[:tj, j * NSLOT : (j + 1) * NSLOT],
                            rhs=hTs[j][:tj, :mc],
                            start=(j == 0),
                            stop=(j == NJ - 1),
                        )
                    nc.vector.tensor_copy(
                        out=out_sb[:, cs : cs + mc], in_=sel[:NSLOT, :mc]
                    )

            if l < 3:
                in_tiles = new_h
                in8 = h8n
                ktin = KT_D

        # ---- epilogue: L2-normalize selected hidden, write out ----
        nq = (D + GW_MAX - 1) // GW_MAX
        ssp = sb.tile([NSLOT, nq], FP32, tag="ssp")
        for q in range(nq):
            q0 = q * GW_MAX
            qw = min(GW_MAX, D - q0)
            pt = pp.tile([P, GW_MAX], FP32, tag="pt", bufs=5, name=f"ptq{q}")
            nc.scalar.activation(
                out=pt[:NSLOT, :qw],
                in_=out_sb[:, q0 : q0 + qw],
                func=AF.Square,
                accum_out=ssp[:, q : q + 1],
            )
        ss = sb.tile([NSLOT, 1], FP32, tag="ss")
        nc.vector.tensor_reduce(
            out=ss[:], in_=ssp[:], axis=mybir.AxisListType.X, op=OP.add
        )
        # 1/sqrt(ss + eps); eps guards empty slots (zero rows)
        eps = sb.tile([NSLOT, 1], FP32, tag="eps")
        nc.gpsimd.memset(eps[:], 1e-20)
        sq = sb.tile([NSLOT, 1], FP32, tag="sq")
        nc.scalar.activation(out=sq[:], in_=ss[:], func=AF.Sqrt, bias=eps[:, 0:1])
        inv = sb.tile([NSLOT, 1], FP32, tag="inv")
        nc.vector.reciprocal(out=inv[:], in_=sq[:])
        nc.vector.tensor_scalar(
            out=out_sb[:], in0=out_sb[:], scalar1=inv[:, 0:1], scalar2=None,
            op0=OP.mult,
        )
        nc.sync.dma_start(out=out_d[:, :], in_=out_sb[:])

    nc.compile()
    return nc


# ---------------------------------------------------------------------------
# Host-side input prep
# ---------------------------------------------------------------------------


def _retile_gates(W, Kin, D, gate_idx, NKD, NKI_l, np_dtype, scale=1.0):
    """Gate blocks of [Kin, gates*D] -> [NKD*ng, 128, nki*128] contiguous."""
    nki = NKI_l
    ng = len(gate_idx)
    Wp = np.zeros((nki * P, ng * NKD * P), np.float32)
    Dp = NKD * P
    src = np.asarray(W, np.float32)
    for gi, g in enumerate(gate_idx):
        Wp[:Kin, gi * Dp : gi * Dp + D] = src[:, g * D : (g + 1) * D] * scale
    # [kt*P+p, gi*Dp + ci*P + m] -> [ci*ng+gi, p, kt*P+m]
    Wp = Wp.reshape(nki, P, ng, NKD, P)
    Wt = np.ascontiguousarray(np.transpose(Wp, (3, 2, 1, 0, 4)))
    return Wt.reshape(NKD * ng, P, nki * P).astype(np_dtype)


def _pack_bias(b_half, D, NKD):
    """[D] -> [128, NKD]: column ci holds channels ci*128..ci*128+127."""
    pad = NKD * P - D
    bp = np.pad(np.asarray(b_half, np.float32), (0, pad))
    return np.ascontiguousarray(bp.reshape(NKD, P).T)


def make_core_inputs(core, plan, x, lengths, embed, Ws, bs, K, D, V):
    NCH, CCH, SMAX = plan.NCH, plan.CCH, plan.SMAX
    NTOT, NSLOT = plan.NTOT, plan.NSLOT
    NTOKP = ((NTOT + P - 1) // P) * P
    NJ = NTOKP // P
    NKD = len(_ptiles(D))

    xl = np.zeros(NTOKP, np.int32)
    bmask = np.zeros((1, NTOT), np.float32)
    onehot = np.zeros((NTOKP, NSLOT), np.float32)
    for ch, bin_seqs in enumerate(plan.bins[core]):
        pos = 0
        for k, b in enumerate(bin_seqs):
            ln = int(lengths[b])
            if ln <= 0:
                continue
            col0 = ch * CCH + pos
            xl[col0 : col0 + ln] = x[b, :ln]
            bmask[0, col0 + 1 : col0 + ln] = 1.0
            onehot[col0 + ln - 1, ch * SMAX + k] = 1.0
            pos += ln

    # [NTOKP, NSLOT] -> [P, NJ*NSLOT]: chunk j partition p = token j*128+p
    oh = np.ascontiguousarray(
        onehot.reshape(NJ, P, NSLOT).transpose(1, 0, 2).reshape(P, NJ * NSLOT)
    ).astype(BF16_NP)

    emb_bf = embed if embed.dtype == BF16_NP else np.asarray(embed).astype(BF16_NP)

    im = {
        "xidx": xl,
        "embed": emb_bf,
        "bmask": np.broadcast_to(bmask, (P, NTOT)).copy(),
        "onehot": oh,
    }
    for l in range(4):
        im[f"W{l}t"] = Ws[l][0]
        im[f"W{l}g8"] = Ws[l][1]
        im[f"bf{l}"] = _pack_bias(bs[l][:D], D, NKD)
        im[f"br{l}"] = _pack_bias(bs[l][D:], D, NKD)
    return im


_NC_CACHE = {}


def kernel(x, lengths, embed, W0, b0, W1, b1, W2, b2, W3, b3):
    x = np.asarray(x)
    lengths = np.asarray(lengths)
    embed = np.asarray(embed, np.float32).astype(BF16_NP)
    Ws = [np.asarray(w, np.float32) for w in (W0, W1, W2, W3)]
    bs = [np.asarray(b, np.float32) for b in (b0, b1, b2, b3)]

    Bb, T = x.shape
    V, K = embed.shape
    D = Ws[1].shape[0]

    plan = make_plan(lengths)
    key = (K, D, V, plan.NCH, plan.CCH, plan.SMAX)
    if key not in _NC_CACHE:
        _NC_CACHE[key] = build_program(*key)
    nc = _NC_CACHE[key]

    NKD = len(_ptiles(D))
    NKI = len(_ptiles(K))
    Wt = []
    for l in range(4):
        Kin = K if l == 0 else D
        nki = NKI if l == 0 else NKD
        bidx = [0, 3] if l == 0 else [0]  # cand (+ highway on l0)
        Wt.append(
            (
                _retile_gates(Ws[l], Kin, D, bidx, NKD, nki, BF16_NP),
                _retile_gates(Ws[l], Kin, D, [1, 2], NKD, nki, FP8_NP, W8S),
            )
        )
    in_maps = [
        make_core_inputs(c, plan, x, lengths, embed, Wt, bs, K, D, V)
        for c in range(N_CORES)
    ]
    res = run_bass_kernel_spmd(nc, in_maps, core_ids=list(range(N_CORES)))

    out = np.zeros((Bb, D), np.float32)
    for c in range(N_CORES):
        oc = res.results[c]["out"]
        for ch, bin_seqs in enumerate(plan.bins[c]):
            for k, b in enumerate(bin_seqs):
                out[b] = oc[ch * plan.SMAX + k]
    return out


# revision 24
# speedup vs baseline: 1.7072x; 1.1145x over previous
"""Trainium2 Bass kernel for nn_EncoderText (4-layer SRU text encoder).

Reference computation:
  e = embed[x]                       # [B, T, K]
  4x SRU layers over time (layer0: k=4 projections incl highway; 1-3: k=3)
  gather last valid timestep per sequence, L2-normalize over features.

Strategy:
- Data-parallel over sequences across 8 NeuronCores, no collectives.
- Only VALID tokens are computed: sequences are LPT-bin-packed into
  NCH chunks of CCH columns per core (all cores share one program
  shape; the assignment of sequences to cores/chunks is host-side).
- Activations are channel-major [D, tokens]; the SRU recurrence
  c_t = f_t*c_{t-1} + (1-f_t)*cand_t maps onto the DVE
  tensor_tensor_scan along the free axis; a boundary mask (runtime
  input) zeroes f at each sequence start so sequences chain safely
  through one scan per column-group.
- Candidate/highway weights+activations in bf16; f/r gate projections
  in fp8e4m3 (weights x64, activations x16, descaled inside the
  sigmoid) using DoubleRow matmuls that contract two k-tiles per
  instruction; recurrence and gates stay fp32.
- Chunks are fused into column groups of <=512 so each matmul streams
  the whole group (fewer, longer matmuls; one PSUM bank per gate).
- Last-timestep selection via PE: transpose the final h tiles and
  multiply with a host-built one-hot matrix [tokens, slot] -> the
  output lands directly as [slot, channels]; slot->sequence mapping is
  undone on the host.
"""

from contextlib import ExitStack

import numpy as np

import concourse.bass as bass
import concourse.mybir as mybir
import concourse.tile as tile
from concourse import bacc
from concourse.bass_utils import run_bass_kernel_spmd
from concourse.masks import make_identity

FP32 = mybir.dt.float32
BF16 = mybir.dt.bfloat16
FP8 = mybir.dt.float8e4
I32 = mybir.dt.int32
AF = mybir.ActivationFunctionType
OP = mybir.AluOpType
DR = mybir.MatmulPerfMode.DoubleRow

BF16_NP = mybir.dt.np(BF16)
FP8_NP = mybir.dt.np(FP8)

P = 128
N_CORES = 8
GW_MAX = 512  # max columns per matmul / PSUM bank (fp32)
W8S = 64.0  # fp8 gate-weight scale
H8S = 16.0  # fp8 gate-activation scale
SIG_SCALE = 1.0 / (W8S * H8S)


def _ptiles(n):
    """[(start, size)] partition tiles of <=128 covering n."""
    out = []
    s = 0
    while s < n:
        out.append((s, min(P, n - s)))
        s += P
    return out


def _groups(NCH, CCH):
    """Fuse chunks into column groups of width <= GW_MAX."""
    cpg = max(1, GW_MAX // CCH)
    out = []
    c = 0
    while c < NCH:
        n = min(cpg, NCH - c)
        out.append((c * CCH, n * CCH))
        c += n
    return out


# ---------------------------------------------------------------------------
# Host-side packing plan
# ---------------------------------------------------------------------------


class Plan:
    def __init__(self, NCH, CCH, SMAX, bins):
        self.NCH = NCH  # chunks per core
        self.CCH = CCH  # columns per chunk
        self.SMAX = SMAX  # max sequences per chunk
        self.bins = bins  # [N_CORES][NCH] -> list of global seq ids
        self.NTOT = NCH * CCH
        self.NSLOT = NCH * SMAX


def make_plan(lengths):
    """LPT bin-pack sequences into N_CORES*NCH bins of capacity CCH."""
    lengths = np.asarray(lengths, np.int64)
    order = np.argsort(-lengths, kind="stable")
    maxlen = int(lengths.max())

    best = None
    for NCH in range(1, 17):
        nbins = N_CORES * NCH
        loads = np.zeros(nbins, np.int64)
        bins = [[] for _ in range(nbins)]
        for i in order:
            j = int(np.argmin(loads))
            loads[j] += int(lengths[i])
            bins[j].append(int(i))
        C = int(loads.max())
        CCH = max(maxlen, -(-C // 8) * 8)
        if CCH > GW_MAX:
            continue
        cost = NCH * CCH
        if best is None or cost < best[0]:
            SMAX = max(len(b) for b in bins)
            best = (cost, NCH, CCH, SMAX, bins)
        if best is not None and NCH * maxlen >= best[0]:
            break
    assert best is not None, "no feasible packing"
    _, NCH, CCH, SMAX, bins = best
    core_bins = [bins[c * NCH : (c + 1) * NCH] for c in range(N_CORES)]
    plan = Plan(NCH, CCH, SMAX, core_bins)
    assert plan.NSLOT <= P, f"too many slots {plan.NSLOT}"
    return plan


# ---------------------------------------------------------------------------
# Device program
# ---------------------------------------------------------------------------


def build_program(K, D, V, NCH, CCH, SMAX):
    """Emit the per-core program.

    DRAM parameters (per core):
      xidx   [NTOKP]      int32  packed token row indices (pad -> 0)
      embed  [V, K]       bf16   full embedding table (replicated)
      W{l}t  [NKD*gates, P, nki*P] bf16  host-pre-tiled weights
      bf{l}, br{l} [128, NKD] f32  per-channel-tile bias columns
      bmask  [128, NTOT]  f32    0 at each sequence start (and padding), else 1
      onehot [128, NJ*NSLOT] bf16  one-hot last-token selector per slot
      out    [NSLOT, D]   f32    normalized last-step hidden per slot
    """
    NTOT = NCH * CCH
    NSLOT = NCH * SMAX
    NTOKP = ((NTOT + P - 1) // P) * P
    NJ = NTOKP // P
    KT_IN = _ptiles(K)
    KT_D = _ptiles(D)
    NKI, NKD = len(KT_IN), len(KT_D)
    TOK_T = _ptiles(NTOT)
    GRP = _groups(NCH, CCH)
    assert max(gw for _, gw in GRP) <= GW_MAX

    nc = bacc.Bacc("TRN2", target_bir_lowering=False, debug=False)

    xidx = nc.declare_dram_parameter("xidx", [NTOKP], I32, isOutput=False)
    emb = nc.declare_dram_parameter("embed", [V, K], BF16, isOutput=False)
    Wd, Wd8 = [], []
    for l in range(4):
        gb = 2 if l == 0 else 1  # bf16 gates: cand (+ highway on l0)
        nki_l = NKI if l == 0 else NKD
        Wd.append(
            nc.declare_dram_parameter(
                f"W{l}t", [NKD * gb, P, nki_l * P], BF16, isOutput=False
            )
        )
        Wd8.append(
            nc.declare_dram_parameter(
                f"W{l}g8", [NKD * 2, P, nki_l * P], FP8, isOutput=False
            )
        )
    bfd = [
        nc.declare_dram_parameter(f"bf{l}", [P, NKD], FP32, isOutput=False)
        for l in range(4)
    ]
    brd = [
        nc.declare_dram_parameter(f"br{l}", [P, NKD], FP32, isOutput=False)
        for l in range(4)
    ]
    bmask_d = nc.declare_dram_parameter("bmask", [P, NTOT], BF16, isOutput=False)
    onehot_d = nc.declare_dram_parameter(
        "onehot", [P, NJ * NSLOT], BF16, isOutput=False
    )
    out_d = nc.declare_dram_parameter("out", [NSLOT, D], FP32, isOutput=True)

    with tile.TileContext(nc) as tc, ExitStack() as ctx:
        sb = ctx.enter_context(tc.tile_pool(name="sb", bufs=1))
        big = ctx.enter_context(tc.tile_pool(name="big", bufs=NKI + 2 * NKD))
        wp = ctx.enter_context(tc.tile_pool(name="wp", bufs=1))
        tp = ctx.enter_context(tc.tile_pool(name="tp", bufs=2))
        pp = ctx.enter_context(tc.tile_pool(name="pp", bufs=1, space="PSUM"))

        # ---- constants ----
        identity = sb.tile([P, P], BF16, tag="identity")
        make_identity(nc, identity[:])
        bmask = sb.tile([P, NTOT], BF16, tag="bmask")
        nc.sync.dma_start(out=bmask[:], in_=bmask_d[:, :])
        onehot = sb.tile([P, NJ * NSLOT], BF16, tag="onehot")
        nc.sync.dma_start(out=onehot[:], in_=onehot_d[:, :])
        bfs, brs = [], []
        for l in range(4):
            bft = sb.tile([P, NKD], FP32, tag=f"bf{l}")
            nc.sync.dma_start(out=bft[:], in_=bfd[l][:, :])
            bfs.append(bft)
            brt = sb.tile([P, NKD], FP32, tag=f"br{l}")
            nc.sync.dma_start(out=brt[:], in_=brd[l][:, :])
            brs.append(brt)
        idx_sb = sb.tile([P, NJ], I32, tag="idx")
        nc.sync.dma_start(
            out=idx_sb[:], in_=xidx[:].rearrange("(j p) -> p j", p=P)
        )

        # ---- embedding gather + transpose to channel-major e_T (bf16+fp8) ----
        eT = []
        for k in range(NKI):
            et = big.tile([P, NTOT], BF16, tag="hb", name=f"eT{k}")
            eT.append(et)
        e8 = sb.tile([P, NKI, NTOT], FP8, tag="e8")
        for j, (ts_, tj) in enumerate(TOK_T):
            eg = sb.tile([P, K], BF16, tag="eg", bufs=2, name=f"eg{j}")
            nc.gpsimd.indirect_dma_start(
                out=eg[:tj, :],
                out_offset=None,
                in_=emb[:, :],
                in_offset=bass.IndirectOffsetOnAxis(
                    ap=idx_sb[:tj, j : j + 1], axis=0
                ),
            )
            for k, (ks, kk) in enumerate(KT_IN):
                pt = pp.tile([P, P], BF16, tag="ptT", bufs=2, name=f"ptr{j}_{k}")
                nc.tensor.transpose(
                    out=pt[:kk, :tj],
                    in_=eg[:tj, ks : ks + kk],
                    identity=identity[:tj, :tj],
                )
                nc.vector.tensor_copy(
                    out=eT[k][:kk, ts_ : ts_ + tj], in_=pt[:kk, :tj]
                )
                nc.scalar.activation(
                    out=e8[:kk, k, ts_ : ts_ + tj], in_=pt[:kk, :tj],
                    func=AF.Copy, scale=H8S,
                )

        # ---- SRU layers ----
        out_sb = sb.tile([NSLOT, D], FP32, tag="out_sb")
        ssp = sb.tile([NSLOT, NKD], FP32, tag="ssp")

        in_tiles = eT
        in8 = e8
        ktin = KT_IN
        for l in range(4):
            gb = 2 if l == 0 else 1
            nki = len(ktin)
            npair = nki // 2
            krem = ktin[-1][1] if nki % 2 else None

            new_h = None
            h8n = None
            if l < 3:
                new_h = [
                    big.tile([P, NTOT], BF16, tag="hb", name=f"h{l}_{ci}")
                    for ci in range(NKD)
                ]
                h8n = tp.tile([P, NKD, NTOT], FP8, tag="h8", bufs=2,
                              name=f"h8_{l}")

            for ci, (cs, mc) in enumerate(KT_D):
                wts = []
                for g in range(gb):
                    wt = wp.tile(
                        [P, nki, P], BF16, tag=f"wb{g}", bufs=3,
                        name=f"w{l}_{ci}_{g}",
                    )
                    nc.sync.dma_start(
                        out=wt[:, :, :],
                        in_=Wd[l][ci * gb + g, :, :].rearrange(
                            "p (kt m) -> p kt m", m=P
                        ),
                    )
                    wts.append(wt)
                w8s = []
                for g in range(2):
                    w8 = wp.tile(
                        [P, nki, P], FP8, tag=f"w8{g}", bufs=3,
                        name=f"w8_{l}_{ci}_{g}",
                    )
                    nc.sync.dma_start(
                        out=w8[:, :, :],
                        in_=Wd8[l][ci * 2 + g, :, :].rearrange(
                            "p (kt m) -> p kt m", m=P
                        ),
                    )
                    w8s.append(w8)

                hh = None
                if l == 3:
                    hh = tp.tile([P, NTOT], BF16, tag="hh", name=f"hh{ci}")

                for gs, gw in GRP:
                    gsl = slice(gs, gs + gw)
                    # bf16 gates: cand (+ highway)
                    ps = []
                    for g in range(gb):
                        pt = pp.tile(
                            [P, GW_MAX], FP32, tag="pt", bufs=5,
                            name=f"pm{l}_{ci}_{g}",
                        )
                        for k2, (ks2, kk2) in enumerate(ktin):
                            nc.tensor.matmul(
                                out=pt[:mc, :gw],
                                lhsT=wts[g][:kk2, k2, :mc],
                                rhs=in_tiles[k2][:kk2, gsl],
                                start=(k2 == 0),
                                stop=(k2 == nki - 1),
                            )
                        ps.append(pt)
                    # fp8 gates: f, r via DoubleRow k-tile pairs
                    ps8 = []
                    for g in range(2):
                        pt = pp.tile(
                            [P, GW_MAX], FP32, tag="pt", bufs=5,
                            name=f"pg{l}_{ci}_{g}",
                        )
                        for j in range(npair):
                            nc.tensor.matmul(
                                out=pt[:mc, :gw],
                                lhsT=w8s[g][:, 2 * j : 2 * j + 2, :mc],
                                rhs=in8[:, 2 * j : 2 * j + 2, gsl],
                                start=(j == 0),
                                stop=(j == npair - 1 and krem is None),
                                perf_mode=DR,
                            )
                        if krem is not None:
                            nc.tensor.matmul(
                                out=pt[:mc, :gw],
                                lhsT=w8s[g][:krem, nki - 1, :mc],
                                rhs=in8[:krem, nki - 1, gsl],
                                start=(npair == 0),
                                stop=True,
                            )
                        ps8.append(pt)
                    cand = ps[0][:mc, :gw]
                    fpre = ps8[0][:mc, :gw]
                    rpre = ps8[1][:mc, :gw]

                    fsb = tp.tile([P, GW_MAX], BF16, tag="fsb", name=f"f{l}_{ci}")
                    nc.scalar.activation(
                        out=fsb[:mc, :gw],
                        in_=fpre,
                        func=AF.Sigmoid,
                        bias=bfs[l][:mc, ci : ci + 1],
                        scale=SIG_SCALE,
                    )
                    rsb = tp.tile([P, GW_MAX], BF16, tag="rsb", name=f"r{l}_{ci}")
                    nc.scalar.activation(
                        out=rsb[:mc, :gw],
                        in_=rpre,
                        func=AF.Sigmoid,
                        bias=brs[l][:mc, ci : ci + 1],
                        scale=SIG_SCALE,
                    )
                    # z' = (f - 1) * cand  == -(1-f)*cand
                    zb = tp.tile([P, GW_MAX], BF16, tag="zb", name=f"z{l}_{ci}")
                    nc.vector.scalar_tensor_tensor(
                        out=zb[:mc, :gw],
                        in0=fsb[:mc, :gw],
                        scalar=1.0,
                        in1=cand,
                        op0=OP.subtract,
                        op1=OP.mult,
                    )
                    # f masked at sequence starts (in place)
                    nc.vector.tensor_mul(
                        out=fsb[:mc, :gw], in0=fsb[:mc, :gw], in1=bmask[:mc, gsl]
                    )
                    # c_t = fm*c_{t-1} - z'
                    cst = tp.tile([P, GW_MAX], BF16, tag="cst", name=f"c{l}_{ci}")
                    nc.vector.tensor_tensor_scan(
                        out=cst[:mc, :gw],
                        data0=fsb[:mc, :gw],
                        data1=zb[:mc, :gw],
                        initial=0.0,
                        op0=OP.mult,
                        op1=OP.subtract,
                    )
                    # h = r*tanh(c) + (1-r)*xres, built in place over cst
                    nc.scalar.activation(
                        out=cst[:mc, :gw], in_=cst[:mc, :gw], func=AF.Tanh
                    )
                    if l == 0:
                        # free the highway PSUM bank early; bf16 SBUF copy
                        # also keeps the sub/add ops in 2x DVE mode
                        hwsb = tp.tile(
                            [P, GW_MAX], BF16, tag="hwsb", name=f"hw{ci}"
                        )
                        nc.scalar.activation(
                            out=hwsb[:mc, :gw], in_=ps[1][:mc, :gw], func=AF.Copy
                        )
                        xres = hwsb[:mc, :gw]
                    else:
                        xres = in_tiles[ci][:mc, gsl]
                    nc.vector.tensor_sub(
                        out=cst[:mc, :gw], in0=cst[:mc, :gw], in1=xres
                    )
                    nc.vector.tensor_mul(
                        out=cst[:mc, :gw], in0=cst[:mc, :gw], in1=rsb[:mc, :gw]
                    )
                    dest = new_h[ci] if l < 3 else hh
                    nc.vector.tensor_add(
                        out=dest[:mc, gsl], in0=cst[:mc, :gw], in1=xres
                    )
                    if l < 3:
                        nc.scalar.activation(
                            out=h8n[:mc, ci, gsl], in_=dest[:mc, gsl],
                            func=AF.Copy, scale=H8S,
                        )

                if l == 3:
                    # last-token selection: sel[slot, ch] = onehot.T @ hh.T
                    hTs = []
                    for j, (ts_, tj) in enumerate(TOK_T):
                        ptj = pp.tile(
                            [P, P], BF16, tag="ptT", bufs=2, name=f"ptT{ci}_{j}"
                        )
                        nc.tensor.transpose(
                            out=ptj[:tj, :mc],
                            in_=hh[:mc, ts_ : ts_ + tj],
                            identity=identity[:mc, :mc],
                        )
                        hT = tp.tile(
                            [P, P], BF16, tag="hT", bufs=NJ, name=f"hT{ci}_{j}"
                        )
                        nc.scalar.activation(
                            out=hT[:tj, :mc], in_=ptj[:tj, :mc], func=AF.Copy
                        )
                        hTs.append(hT)
                    sel = pp.tile(
                        [P, P], FP32, tag="sel", bufs=1, name=f"sel{ci}"
                    )
                    for j, (ts_, tj) in enumerate(TOK_T):
                        nc.tensor.matmul(
                            out=sel[:NSLOT, :mc],
                            lhsT=onehot[:tj, j * NSLOT : (j + 1) * NSLOT],
                            rhs=hTs[j][:tj, :mc],
                            start=(j == 0),
                            stop=(j == NJ - 1),
                        )
                    nc.vector.tensor_copy(
                        out=out_sb[:, cs : cs + mc], in_=sel[:NSLOT, :mc]
                    )
                    # per-tile sum of squares (overlaps with later tiles)
                    sqo = tp.tile([P, P], FP32, tag="sqo", name=f"sqo{ci}")
                    nc.scalar.activation(
                        out=sqo[:NSLOT, :mc],
                        in_=out_sb[:, cs : cs + mc],
                        func=AF.Square,
                        accum_out=ssp[:, ci : ci + 1],
                    )

            if l < 3:
                in_tiles = new_h
                in8 = h8n
                ktin = KT_D

        # ---- epilogue: L2-normalize selected hidden, write out ----
        ss = sb.tile([NSLOT, 1], FP32, tag="ss")
        nc.vector.tensor_reduce(
            out=ss[:], in_=ssp[:], axis=mybir.AxisListType.X, op=OP.add
        )
        # 1/sqrt(ss + eps); eps guards empty slots (zero rows)
        eps = sb.tile([NSLOT, 1], FP32, tag="eps")
        nc.gpsimd.memset(eps[:], 1e-20)
        sq = sb.tile([NSLOT, 1], FP32, tag="sq")
        nc.scalar.activation(out=sq[:], in_=ss[:], func=AF.Sqrt, bias=eps[:, 0:1])
        inv = sb.tile([NSLOT, 1], FP32, tag="inv")
        nc.vector.reciprocal(out=inv[:], in_=sq[:])
        nc.vector.tensor_scalar(
            out=out_sb[:], in0=out_sb[:], scalar1=inv[:, 0:1], scalar2=None,
            op0=OP.mult,
        )
        nc.sync.dma_start(out=out_d[:, :], in_=out_sb[:])

    nc.compile()
    return nc


# ---------------------------------------------------------------------------
# Host-side input prep
# ---------------------------------------------------------------------------


def _retile_gates(W, Kin, D, gate_idx, NKD, NKI_l, np_dtype, scale=1.0):
    """Gate blocks of [Kin, gates*D] -> [NKD*ng, 128, nki*128] contiguous."""
    nki = NKI_l
    ng = len(gate_idx)
    Wp = np.zeros((nki * P, ng * NKD * P), np.float32)
    Dp = NKD * P
    src = np.asarray(W, np.float32)
    for gi, g in enumerate(gate_idx):
        Wp[:Kin, gi * Dp : gi * Dp + D] = src[:, g * D : (g + 1) * D] * scale
    # [kt*P+p, gi*Dp + ci*P + m] -> [ci*ng+gi, p, kt*P+m]
    Wp = Wp.reshape(nki, P, ng, NKD, P)
    Wt = np.ascontiguousarray(np.transpose(Wp, (3, 2, 1, 0, 4)))
    return Wt.reshape(NKD * ng, P, nki * P).astype(np_dtype)


def _pack_bias(b_half, D, NKD):
    """[D] -> [128, NKD]: column ci holds channels ci*128..ci*128+127."""
    pad = NKD * P - D
    bp = np.pad(np.asarray(b_half, np.float32), (0, pad))
    return np.ascontiguousarray(bp.reshape(NKD, P).T)


def make_core_inputs(core, plan, x, lengths, embed, Ws, bs, K, D, V):
    NCH, CCH, SMAX = plan.NCH, plan.CCH, plan.SMAX
    NTOT, NSLOT = plan.NTOT, plan.NSLOT
    NTOKP = ((NTOT + P - 1) // P) * P
    NJ = NTOKP // P
    NKD = len(_ptiles(D))

    xl = np.zeros(NTOKP, np.int32)
    bmask = np.zeros((1, NTOT), np.float32)
    onehot = np.zeros((NTOKP, NSLOT), np.float32)
    for ch, bin_seqs in enumerate(plan.bins[core]):
        pos = 0
        for k, b in enumerate(bin_seqs):
            ln = int(lengths[b])
            if ln <= 0:
                continue
            col0 = ch * CCH + pos
            xl[col0 : col0 + ln] = x[b, :ln]
            bmask[0, col0 + 1 : col0 + ln] = 1.0
            onehot[col0 + ln - 1, ch * SMAX + k] = 1.0
            pos += ln

    # [NTOKP, NSLOT] -> [P, NJ*NSLOT]: chunk j partition p = token j*128+p
    oh = np.ascontiguousarray(
        onehot.reshape(NJ, P, NSLOT).transpose(1, 0, 2).reshape(P, NJ * NSLOT)
    ).astype(BF16_NP)

    emb_bf = embed if embed.dtype == BF16_NP else np.asarray(embed).astype(BF16_NP)

    im = {
        "xidx": xl,
        "embed": emb_bf,
        "bmask": np.broadcast_to(bmask, (P, NTOT)).astype(BF16_NP),
        "onehot": oh,
    }
    for l in range(4):
        im[f"W{l}t"] = Ws[l][0]
        im[f"W{l}g8"] = Ws[l][1]
        im[f"bf{l}"] = _pack_bias(bs[l][:D], D, NKD)
        im[f"br{l}"] = _pack_bias(bs[l][D:], D, NKD)
    return im


_NC_CACHE = {}


def kernel(x, lengths, embed, W0, b0, W1, b1, W2, b2, W3, b3):
    x = np.asarray(x)
    lengths = np.asarray(lengths)
    embed = np.asarray(embed, np.float32).astype(BF16_NP)
    Ws = [np.asarray(w, np.float32) for w in (W0, W1, W2, W3)]
    bs = [np.asarray(b, np.float32) for b in (b0, b1, b2, b3)]

    Bb, T = x.shape
    V, K = embed.shape
    D = Ws[1].shape[0]

    plan = make_plan(lengths)
    key = (K, D, V, plan.NCH, plan.CCH, plan.SMAX)
    if key not in _NC_CACHE:
        _NC_CACHE[key] = build_program(*key)
    nc = _NC_CACHE[key]

    NKD = len(_ptiles(D))
    NKI = len(_ptiles(K))
    Wt = []
    for l in range(4):
        Kin = K if l == 0 else D
        nki = NKI if l == 0 else NKD
        bidx = [0, 3] if l == 0 else [0]  # cand (+ highway on l0)
        Wt.append(
            (
                _retile_gates(Ws[l], Kin, D, bidx, NKD, nki, BF16_NP),
                _retile_gates(Ws[l], Kin, D, [1, 2], NKD, nki, FP8_NP, W8S),
            )
        )
    in_maps = [
        make_core_inputs(c, plan, x, lengths, embed, Wt, bs, K, D, V)
        for c in range(N_CORES)
    ]
    res = run_bass_kernel_spmd(nc, in_maps, core_ids=list(range(N_CORES)))

    out = np.zeros((Bb, D), np.float32)
    for c in range(N_CORES):
        oc = res.results[c]["out"]
        for ch, bin_seqs in enumerate(plan.bins[c]):
            for k, b in enumerate(bin_seqs):
                out[b] = oc[ch * plan.SMAX + k]
    return out
